# revision 31
# baseline (speedup 1.0000x reference)
# Trainium2 Bass kernel for nn_FFF_v2 (fast-feedforward / MoE tree routing).
#
#   lam   = x @ W.T                      [B, 12] router logits
#   branch= lam > 0                      tree descent decisions
#   node  = (2^i - 1) + sum_{j<i} branch_j 2^(i-1-j)
#   out   = sum_i lam_i * Y[node_i]      [B, 4096]
#
# Sharding: data-parallel on batch across 8 cores (1024 rows each); W and Y
# replicated.  Per core:
#   - router logits via fp16 hi/lo 3-pass PE matmul (sign-exact vs the f32
#     reference: residual ~2^-22; a flipped branch would rewrite deeper node
#     ids and blow the 2e-2 absmax gate, so plain fp16/bf16 is NOT enough)
#   - tree-node ids via small exact fp16 matmuls (powers-of-two weights;
#     fp16 is integer-exact through 2048 > max node id offset)
#   - levels 0..K_MM-1: scaled-one-hot fp16 matmul against SBUF-resident
#     shallow Y rows; levels K_MM..11: dma_gather of fp16 Y rows from HBM,
#     scaled in-place on the vector engine (tensor_scalar, 4x mode) and
#     summed with fp16 tensor_tensor adds (2x mode)
#   - software pipeline: the back half (one-hot + scales + sums + store) of
#     subtile s is emitted after the front half (router share + indices +
#     gather launch) of subtile s+1, so no engine stream head-of-line blocks
#     on a cross-engine dependency
#   - out written fp16, upcast to f32 on host (2e-2 gate leaves ~25x margin)
import numpy as np

DEPTH = 12
B = 8192
D = 4096
N_NODES = 4095
NCORES = 8
B_LOC = B // NCORES          # 1024 rows per core

MACRO = 256                  # batch rows per macro tile
SUB = 128                    # rows per subtile (one partition block)
NSUB = MACRO // SUB          # 2
NMACRO = B_LOC // MACRO      # 4

K_MM = 9                     # levels 0..K_MM-1 handled by one-hot matmul
N_SH = 2 ** K_MM - 1         # shallow nodes (511)
NCHUNK = (N_SH + 127) // 128  # 4
N_GL = DEPTH - K_MM          # gather levels (3)

PIPE = 2                     # back-phase delay in subtiles (one full macro)

_CACHE = {}


def _level_of(n):
    # level i spans nodes [2^i - 1, 2^(i+1) - 1)
    lev = 0
    while n >= 2 ** (lev + 1) - 1:
        lev += 1
    return lev


def _host_consts():
    # powT[j, i] = powmat[i, j] = 2^(i-1-j) for j < i  (lhsT of prefix matmul)
    powT = np.zeros((DEPTH, DEPTH), np.float16)
    for i in range(DEPTH):
        for j in range(i):
            powT[j, i] = float(1 << (i - 1 - j))
    # offs_w[p, l*8+f] = 2^l - 1 (level offset in wrapped (level, slot) layout)
    offs = np.array([(1 << i) - 1 for i in range(DEPTH)], np.float32)
    offs_w = np.broadcast_to(
        np.repeat(offs, SUB // 16)[None, :], (16, DEPTH * (SUB // 16))
    ).copy()
    # bselT[l, c*128+p] = 1 if level(c*128+p) == l else 0   (lhsT of bc matmul)
    bselT = np.zeros((DEPTH, NCHUNK * 128), np.float16)
    # nrel[p, c] = node - (2^level - 1), or -1 for pad positions
    nrel = np.full((128, NCHUNK), -1.0, np.float32)
    for c in range(NCHUNK):
        for p in range(128):
            n = c * 128 + p
            if n < N_SH:
                lev = _level_of(n)
                bselT[lev, c * 128 + p] = 1.0
                nrel[p, c] = float(n - ((1 << lev) - 1))
    # identity for PE transpose of the lam/prefix tiles
    ident = np.eye(2 * DEPTH, dtype=np.float16)
    return powT, offs_w, bselT, nrel, ident


def _build_program():
    import concourse.bass as bass
    import concourse.bacc as bacc
    import concourse.mybir as mybir
    import concourse.tile as tile
    from contextlib import ExitStack

    dt = mybir.dt
    f32 = dt.float32
    f16 = dt.float16
    i16 = dt.int16
    Alu = mybir.AluOpType

    nc = bacc.Bacc(trn_type="TRN2", num_swdge_queues=4)

    ydt = f16                 # Y / gather / one-hot / output dtype
    tdt = f16                 # tree-index matmul dtype (integer-exact <= 2048)
    xth_d = nc.dram_tensor("xth", [NMACRO, 128, 32, MACRO], f16, kind="ExternalInput")
    xtl_d = nc.dram_tensor("xtl", [NMACRO, 128, 32, MACRO], f16, kind="ExternalInput")
    y_d = nc.dram_tensor("y", [N_NODES, D], ydt, kind="ExternalInput")
    wth_d = nc.dram_tensor("wth", [128, 32, DEPTH], f16, kind="ExternalInput")
    wtl_d = nc.dram_tensor("wtl", [128, 32, DEPTH], f16, kind="ExternalInput")
    powt_d = nc.dram_tensor("powt", [DEPTH, DEPTH], tdt, kind="ExternalInput")
    offsw_d = nc.dram_tensor(
        "offsw", [16, DEPTH * (SUB // 16)], f32, kind="ExternalInput"
    )
    bselt_d = nc.dram_tensor("bselt", [DEPTH, NCHUNK * 128], tdt, kind="ExternalInput")
    nrel_d = nc.dram_tensor("nrel", [128, NCHUNK], f32, kind="ExternalInput")
    ident_d = nc.dram_tensor("ident", [2 * DEPTH, 2 * DEPTH], tdt, kind="ExternalInput")
    out_d = nc.dram_tensor("out", [B_LOC, D], ydt, kind="ExternalOutput")

    with tile.TileContext(nc) as tc, ExitStack() as ctx:
        consts = ctx.enter_context(tc.tile_pool(name="consts", bufs=1))
        xt_p = ctx.enter_context(tc.tile_pool(name="xt", bufs=2))
        xtl_p = ctx.enter_context(tc.tile_pool(name="xtl", bufs=1))
        small = ctx.enter_context(tc.tile_pool(name="small", bufs=3))
        small4 = ctx.enter_context(tc.tile_pool(name="small4", bufs=8))
        st_p = ctx.enter_context(tc.tile_pool(name="st", bufs=2 * NCHUNK))
        g_p = ctx.enter_context(tc.tile_pool(name="g", bufs=3 * (PIPE + 1)))
        hs_p = ctx.enter_context(tc.tile_pool(name="hs", bufs=2))
        oa_p = ctx.enter_context(tc.tile_pool(name="oa", bufs=2))
        ps_lam = ctx.enter_context(tc.tile_pool(name="pslam", bufs=2, space="PSUM"))
        ps_pb = ps_lam
        ps_bc = ctx.enter_context(tc.tile_pool(name="psbc", bufs=2, space="PSUM"))
        ps_tp = ctx.enter_context(tc.tile_pool(name="pstp", bufs=1, space="PSUM"))
        ps_out = ctx.enter_context(tc.tile_pool(name="psout", bufs=3, space="PSUM"))

        # ---- critical-path constants: router weights only ----
        wth_sb = consts.tile([128, 32, DEPTH], f16)
        nc.sync.dma_start(wth_sb[:], wth_d.ap())
        wtl_sb = consts.tile([128, 32, DEPTH], f16)
        nc.scalar.dma_start(wtl_sb[:], wtl_d.ap())

        state = {}

        def back_phase(mm, ss, gt, lamb, st_list):
            # in-place per-level scaling on the gathered rows (DVE 4x mode)
            for li in range(N_GL):
                nc.vector.tensor_scalar(
                    gt[li][:, 0, :], gt[li][:, 0, :],
                    lamb[:, K_MM + li : K_MM + li + 1], None, Alu.mult,
                )
            # deep-level sum (fp16 TT, 2x mode)
            hs = hs_p.tile([SUB, D], ydt, tag="hs")
            nc.vector.tensor_tensor(hs[:], gt[0][:, 0, :], gt[1][:, 0, :], Alu.add)
            nc.vector.tensor_tensor(hs[:], hs[:], gt[2][:, 0, :], Alu.add)

            # one-hot matmul: shallow-level contribution, drained by ACT
            bsl = slice(ss * SUB, (ss + 1) * SUB)
            oacc = oa_p.tile([SUB, D], ydt, tag="oacc")
            ysh_sb = state["ysh"]
            for q in range(D // 512):
                qsl = slice(q * 512, (q + 1) * 512)
                po = ps_out.tile([SUB, 512], f32, tag="po")
                for c in range(NCHUNK):
                    nc.tensor.matmul(
                        po[:], st_list[c][:, bsl],
                        ysh_sb[:, c * D + q * 512 : c * D + (q + 1) * 512],
                        start=(c == 0), stop=(c == NCHUNK - 1),
                    )
                nc.scalar.copy(oacc[:, qsl], po[:])

            # final sum in-place into hs, then store
            nc.vector.tensor_tensor(hs[:], oacc[:], hs[:], Alu.add)
            nc.scalar.dma_start(
                out_d.ap()[mm * MACRO + ss * SUB :][:SUB, :], hs[:]
            )

        pending = []

        for m in range(NMACRO):
            # ---- load x^T macro tiles (hi+lo fp16), split across rails ----
            xth = xt_p.tile([128, 32, MACRO], f16, tag="xth")
            # both x streams ride the ACT-HWDGE rail: the SP-HWDGE rail is
            # reserved for the small latency-critical idxr replicates, which
            # otherwise queue ahead of these 1MB loads and stall the router
            nc.scalar.dma_start(xth[:, :16, :], xth_d.ap()[m][:, :16, :])
            nc.scalar.dma_start(xth[:, 16:, :], xth_d.ap()[m][:, 16:, :])
            xtl = xtl_p.tile([128, 32, MACRO], f16, tag="xtl")
            nc.scalar.dma_start(xtl[:, :16, :], xtl_d.ap()[m][:, :16, :])
            nc.scalar.dma_start(xtl[:, 16:, :], xtl_d.ap()[m][:, 16:, :])

            # ---- router: lam^T [12, MACRO] = W @ x^T (3-pass fp16 hi/lo) ----
            lam_ps = ps_lam.tile([DEPTH, MACRO], f32, tag="lam")
            passes = [(wth_sb, xth), (wth_sb, xtl), (wtl_sb, xth)]
            for p, (wsb, xsb) in enumerate(passes):
                for c in range(32):
                    nc.tensor.matmul(
                        lam_ps[:], wsb[:, c, :], xsb[:, c, :],
                        start=(p == 0 and c == 0), stop=(p == 2 and c == 31),
                    )

            if m == 0:
                # remaining constants, deferred so the first x tile and the
                # router aren't stuck behind the const DMA traffic
                powt_sb = consts.tile([DEPTH, DEPTH], tdt)
                nc.sync.dma_start(powt_sb[:], powt_d.ap())
                offsw_sb = consts.tile([16, DEPTH * (SUB // 16)], f32)
                nc.sync.dma_start(offsw_sb[:], offsw_d.ap())
                bselt_sb = consts.tile([DEPTH, NCHUNK * 128], tdt)
                nc.sync.dma_start(bselt_sb[:], bselt_d.ap())
                nrel_sb = consts.tile([128, NCHUNK], f32)
                nc.sync.dma_start(nrel_sb[:], nrel_d.ap())
                ident_sb = consts.tile([2 * DEPTH, 2 * DEPTH], tdt)
                nc.sync.dma_start(ident_sb[:], ident_d.ap())
                ysh_sb = consts.tile([128, NCHUNK * D], ydt)
                state["ysh"] = ysh_sb
                for c in range(NCHUNK):
                    # full 128 rows: pad slots (>= N_SH) hold valid-but-
                    # unselected Y rows, never NaN-able garbage SBUF
                    lo = c * 128
                    hi = min(lo + 128, N_NODES)
                    nc.scalar.dma_start(
                        ysh_sb[: hi - lo, c * D : (c + 1) * D], y_d.ap()[lo:hi, :]
                    )

            # branch bits, lam^T and prefix^T in SBUF (partition 0 based)
            branch = small.tile([DEPTH, MACRO], tdt, tag="branch")
            nc.vector.tensor_scalar(branch[:], lam_ps[:], 0.0, None, Alu.is_gt)
            lamT = small.tile([DEPTH, MACRO], tdt, tag="lamT")
            nc.scalar.copy(lamT[:], lam_ps[:])

            # prefix^T [12, MACRO] = powmat @ branch  (exact in fp16)
            pb_ps = ps_pb.tile([DEPTH, MACRO], f32, tag="lam")
            nc.tensor.matmul(pb_ps[:], powt_sb[:], branch[:], start=True, stop=True)
            pfxT = small.tile([DEPTH, MACRO], tdt, tag="pfxT")
            nc.scalar.copy(pfxT[:], pb_ps[:])

            # ---- S^T build: one chunk of 128 shallow nodes at a time ----
            st_list = []
            for c in range(NCHUNK):
                bc_ps = ps_bc.tile([128, 2 * MACRO], f32, tag="bc")
                nc.tensor.matmul(
                    bc_ps[:, :MACRO], bselt_sb[:, c * 128 : (c + 1) * 128],
                    pfxT[:], start=True, stop=True,
                )
                nc.tensor.matmul(
                    bc_ps[:, MACRO:], bselt_sb[:, c * 128 : (c + 1) * 128],
                    lamT[:], start=True, stop=True,
                )
                lbc = small.tile([128, MACRO], ydt, tag="lbc")
                nc.scalar.copy(lbc[:], bc_ps[:, MACRO:])
                stc = st_p.tile([128, MACRO], ydt, tag="st")
                nc.vector.scalar_tensor_tensor(
                    stc[:], bc_ps[:, :MACRO], nrel_sb[:, c : c + 1], lbc[:],
                    Alu.is_equal, Alu.mult,
                )
                st_list.append(stc)

            for s in range(NSUB):
                bsl = slice(s * SUB, (s + 1) * SUB)
                # ---- lam to batch-partition layout ----
                # (plain identity matmul: out = in.T @ I)
                tpw = ps_tp.tile([SUB, 128], f32, tag="tpw")
                tp_ps = tpw[:, :DEPTH]
                nc.tensor.matmul(
                    tp_ps, lamT[:, bsl], ident_sb[:DEPTH, :DEPTH],
                    start=True, stop=True,
                )
                lamb = small4.tile([SUB, DEPTH], f32, tag="lamb")
                nc.vector.tensor_copy(lamb[:], tp_ps)

                # ---- node ids in the 16-partition-wrapped (level, slot)
                # layout dma_gather wants, via per-16-column PE transposes ----
                w_ps = tpw[:16, 16 : 16 + (SUB // 16) * DEPTH].rearrange(
                    "p (f l) -> p f l", f=SUB // 16
                )
                for f in range(SUB // 16):
                    nc.tensor.matmul(
                        w_ps[:, f, :],
                        pfxT[:, s * SUB + f * 16 : s * SUB + (f + 1) * 16],
                        ident_sb[:DEPTH, :DEPTH],
                        start=True, stop=True,
                    )
                idx16 = small4.tile([16, DEPTH, SUB // 16], i16, tag="idx16")
                nc.vector.tensor_tensor(
                    idx16[:], w_ps[:].rearrange("p f l -> p l f"), offsw_sb[:],
                    Alu.add,
                )
                # replicate to all 8 Q7 descriptor-gen cores (SBUF -> SBUF)
                idxr = small4.tile([128, N_GL, SUB // 16], i16, tag="idxr")
                for gq in range(8):
                    nc.sync.dma_start(
                        idxr[16 * gq : 16 * (gq + 1), :, :].rearrange(
                            "p l f -> p (l f)"
                        ),
                        idx16[:, K_MM:, :].rearrange("p l f -> p (l f)"),
                    )

                # ---- gather deep levels from HBM ----
                gt = []
                for li in range(N_GL):
                    g = g_p.tile([128, 1, D], ydt, tag="g")
                    nc.gpsimd.dma_gather(
                        g[:], y_d.ap(), idxr[:, li, :], SUB, SUB, D,
                        queue_num=li % 4,
                    )
                    gt.append(g)

                pending.append((m, s, gt, lamb, st_list))
                while len(pending) > PIPE:
                    back_phase(*pending.pop(0))

        while pending:
            back_phase(*pending.pop(0))

    nc.compile()
    return nc


def _patch_walrus_passes():
    # The default walrus pass list in this environment omits
    # lower_custom_kernel, which the Pool custom instructions (dma_gather)
    # need. Inject it in front of codegen.
    import concourse.bass_utils as bu

    if getattr(bu, "_ant_lck_patched", False):
        return
    bu._ant_lck_patched = True
    orig = bu.run_command

    def run_command(argv, **kw):
        if argv and "walrus_driver" in str(argv[0]):
            argv = list(argv)
            for i, a in enumerate(argv):
                if a == "--pass" and "lower_custom_kernel" not in argv[i + 1]:
                    argv[i + 1] = argv[i + 1].replace(
                        "codegen", "lower_custom_kernel,codegen"
                    )
                    break
        return orig(argv, **kw)

    bu.run_command = run_command


def _get_program():
    if "nc" not in _CACHE:
        _CACHE["nc"] = _build_program()
    return _CACHE["nc"]


def _prep_in_maps(x, W, Y):
    powT, offs_w, bselT, nrel, ident = _host_consts()
    Y16 = np.ascontiguousarray(Y.astype(np.float16))
    wt = W.T.reshape(32, 128, DEPTH).transpose(1, 0, 2)  # [128, 32, 12] f32
    wth = wt.astype(np.float16)
    wtl = (wt - wth.astype(np.float32)).astype(np.float16)
    wth = np.ascontiguousarray(wth)
    wtl = np.ascontiguousarray(wtl)
    in_maps = []
    xr = x.reshape(NCORES, B_LOC, D)
    for c in range(NCORES):
        xt = xr[c].T  # [D, B_LOC]
        xtm = xt.reshape(32, 128, NMACRO, MACRO).transpose(2, 1, 0, 3)
        xth = xtm.astype(np.float16)
        xtl = (xtm - xth.astype(np.float32)).astype(np.float16)
        in_maps.append(
            {
                "xth": np.ascontiguousarray(xth),
                "xtl": np.ascontiguousarray(xtl),
                "y": Y16, "wth": wth, "wtl": wtl, "powt": powT,
                "offsw": offs_w, "bselt": bselT, "nrel": nrel, "ident": ident,
            }
        )
    return in_maps


def kernel(x, W, Y, _trace=False):
    from concourse.bass_utils import run_bass_kernel_spmd

    _patch_walrus_passes()

    nc = _get_program()
    in_maps = _prep_in_maps(np.asarray(x), np.asarray(W), np.asarray(Y))
    res = run_bass_kernel_spmd(nc, in_maps, list(range(NCORES)), trace=_trace)
    out = np.concatenate(
        [res.results[c]["out"].astype(np.float32) for c in range(NCORES)], axis=0
    )
    if _trace:
        _CACHE["last_result"] = res
    return out


# revision 33
# speedup vs baseline: 1.1234x; 1.1234x over previous
# Trainium2 Bass kernel for nn_FFF_v2 (fast-feedforward / MoE tree routing).
#
#   lam   = x @ W.T                      [B, 12] router logits
#   branch= lam > 0                      tree descent decisions
#   node  = (2^i - 1) + sum_{j<i} branch_j 2^(i-1-j)
#   out   = sum_i lam_i * Y[node_i]      [B, 4096]
#
# Sharding: data-parallel on batch across 8 cores (1024 rows each); W and Y
# replicated.  Per core:
#   - router logits via fp16 hi/lo 3-pass PE matmul (sign-exact vs the f32
#     reference: residual ~2^-22; a flipped branch would rewrite deeper node
#     ids and blow the 2e-2 absmax gate, so plain fp16/bf16 is NOT enough)
#   - tree-node ids via small exact fp16 matmuls (powers-of-two weights;
#     fp16 is integer-exact through 2048 > max node id offset)
#   - levels 0..K_MM-1: scaled-one-hot fp16 matmul against SBUF-resident
#     shallow Y rows; levels K_MM..11: dma_gather of fp16 Y rows from HBM,
#     scaled in-place on the vector engine (tensor_scalar, 4x mode) and
#     summed with fp16 tensor_tensor adds (2x mode)
#   - software pipeline: the back half (one-hot + scales + sums + store) of
#     subtile s is emitted after the front half (router share + indices +
#     gather launch) of subtile s+1, so no engine stream head-of-line blocks
#     on a cross-engine dependency
#   - out written fp16, upcast to f32 on host (2e-2 gate leaves ~25x margin)
import numpy as np

DEPTH = 12
B = 8192
D = 4096
N_NODES = 4095
NCORES = 8
B_LOC = B // NCORES          # 1024 rows per core

MACRO = 512                  # batch rows per macro tile
SUB = 128                    # rows per subtile (one partition block)
NSUB = MACRO // SUB          # 2
NMACRO = B_LOC // MACRO      # 4

K_MM = 9                     # levels 0..K_MM-1 handled by one-hot matmul
N_SH = 2 ** K_MM - 1         # shallow nodes (511)
NCHUNK = (N_SH + 127) // 128  # 4
N_GL = DEPTH - K_MM          # gather levels (3)

PIPE = 1                     # back-phase delay in subtiles

_CACHE = {}


def _level_of(n):
    # level i spans nodes [2^i - 1, 2^(i+1) - 1)
    lev = 0
    while n >= 2 ** (lev + 1) - 1:
        lev += 1
    return lev


def _host_consts():
    # powT[j, i] = powmat[i, j] = 2^(i-1-j) for j < i  (lhsT of prefix matmul)
    powT = np.zeros((DEPTH, DEPTH), np.float16)
    for i in range(DEPTH):
        for j in range(i):
            powT[j, i] = float(1 << (i - 1 - j))
    # offs_w[p, l*8+f] = 2^l - 1 (level offset in wrapped (level, slot) layout)
    offs = np.array([(1 << i) - 1 for i in range(DEPTH)], np.float32)
    offs_w = np.broadcast_to(
        np.repeat(offs, SUB // 16)[None, :], (16, DEPTH * (SUB // 16))
    ).copy()
    # bselT[l, c*128+p] = 1 if level(c*128+p) == l else 0   (lhsT of bc matmul)
    bselT = np.zeros((DEPTH, NCHUNK * 128), np.float16)
    # nrel[p, c] = node - (2^level - 1), or -1 for pad positions
    nrel = np.full((128, NCHUNK), -1.0, np.float32)
    for c in range(NCHUNK):
        for p in range(128):
            n = c * 128 + p
            if n < N_SH:
                lev = _level_of(n)
                bselT[lev, c * 128 + p] = 1.0
                nrel[p, c] = float(n - ((1 << lev) - 1))
    # identity for PE transpose of the lam/prefix tiles
    ident = np.eye(2 * DEPTH, dtype=np.float16)
    return powT, offs_w, bselT, nrel, ident


def _build_program():
    import concourse.bass as bass
    import concourse.bacc as bacc
    import concourse.mybir as mybir
    import concourse.tile as tile
    from contextlib import ExitStack

    dt = mybir.dt
    f32 = dt.float32
    f16 = dt.float16
    i16 = dt.int16
    Alu = mybir.AluOpType

    nc = bacc.Bacc(trn_type="TRN2", num_swdge_queues=4)

    ydt = f16                 # Y / gather / one-hot / output dtype
    tdt = f16                 # tree-index matmul dtype (integer-exact <= 2048)
    xth_d = nc.dram_tensor("xth", [NMACRO, 128, 32, MACRO], f16, kind="ExternalInput")
    xtl_d = nc.dram_tensor("xtl", [NMACRO, 128, 32, MACRO], f16, kind="ExternalInput")
    y_d = nc.dram_tensor("y", [N_NODES, D], ydt, kind="ExternalInput")
    wth_d = nc.dram_tensor("wth", [128, 32, DEPTH], f16, kind="ExternalInput")
    wtl_d = nc.dram_tensor("wtl", [128, 32, DEPTH], f16, kind="ExternalInput")
    powt_d = nc.dram_tensor("powt", [DEPTH, DEPTH], tdt, kind="ExternalInput")
    offsw_d = nc.dram_tensor(
        "offsw", [16, DEPTH * (SUB // 16)], f32, kind="ExternalInput"
    )
    bselt_d = nc.dram_tensor("bselt", [DEPTH, NCHUNK * 128], tdt, kind="ExternalInput")
    nrel_d = nc.dram_tensor("nrel", [128, NCHUNK], f32, kind="ExternalInput")
    ident_d = nc.dram_tensor("ident", [2 * DEPTH, 2 * DEPTH], tdt, kind="ExternalInput")
    out_d = nc.dram_tensor("out", [B_LOC, D], ydt, kind="ExternalOutput")

    with tile.TileContext(nc) as tc, ExitStack() as ctx:
        consts = ctx.enter_context(tc.tile_pool(name="consts", bufs=1))
        xt_p = ctx.enter_context(tc.tile_pool(name="xt", bufs=1))
        xtl_p = ctx.enter_context(tc.tile_pool(name="xtl", bufs=1))
        small = ctx.enter_context(tc.tile_pool(name="small", bufs=3))
        small4 = ctx.enter_context(tc.tile_pool(name="small4", bufs=8))
        st_p = ctx.enter_context(tc.tile_pool(name="st", bufs=2 * NCHUNK))
        g_p = ctx.enter_context(tc.tile_pool(name="g", bufs=3 * (PIPE + 1)))
        hs_p = ctx.enter_context(tc.tile_pool(name="hs", bufs=2))
        oa_p = ctx.enter_context(tc.tile_pool(name="oa", bufs=2))
        ps_lam = ctx.enter_context(tc.tile_pool(name="pslam", bufs=2, space="PSUM"))
        ps_pb = ps_lam
        ps_bc = ctx.enter_context(tc.tile_pool(name="psbc", bufs=1, space="PSUM"))
        ps_tp = ctx.enter_context(tc.tile_pool(name="pstp", bufs=1, space="PSUM"))
        ps_out = ctx.enter_context(tc.tile_pool(name="psout", bufs=3, space="PSUM"))

        # ---- critical-path constants: router weights only ----
        wth_sb = consts.tile([128, 32, DEPTH], f16)
        nc.sync.dma_start(wth_sb[:], wth_d.ap())
        wtl_sb = consts.tile([128, 32, DEPTH], f16)
        nc.scalar.dma_start(wtl_sb[:], wtl_d.ap())

        state = {}

        def back_phase(mm, ss, gt, lamb, st_list):
            # in-place per-level scaling on the gathered rows (DVE 4x mode)
            for li in range(N_GL):
                nc.vector.tensor_scalar(
                    gt[li][:, 0, :], gt[li][:, 0, :],
                    lamb[:, K_MM + li : K_MM + li + 1], None, Alu.mult,
                )
            # deep-level sum (fp16 TT, 2x mode)
            hs = hs_p.tile([SUB, D], ydt, tag="hs")
            nc.vector.tensor_tensor(hs[:], gt[0][:, 0, :], gt[1][:, 0, :], Alu.add)
            nc.vector.tensor_tensor(hs[:], hs[:], gt[2][:, 0, :], Alu.add)

            # one-hot matmul: shallow-level contribution, drained by ACT
            bsl = slice(ss * SUB, (ss + 1) * SUB)
            oacc = oa_p.tile([SUB, D], ydt, tag="oacc")
            ysh_sb = state["ysh"]
            for q in range(D // 512):
                qsl = slice(q * 512, (q + 1) * 512)
                po = ps_out.tile([SUB, 512], f32, tag="po")
                for c in range(NCHUNK):
                    nc.tensor.matmul(
                        po[:], st_list[c][:, bsl],
                        ysh_sb[:, c * D + q * 512 : c * D + (q + 1) * 512],
                        start=(c == 0), stop=(c == NCHUNK - 1),
                    )
                nc.scalar.copy(oacc[:, qsl], po[:])

            # final sum in-place into hs, then store
            nc.vector.tensor_tensor(hs[:], oacc[:], hs[:], Alu.add)
            nc.scalar.dma_start(
                out_d.ap()[mm * MACRO + ss * SUB :][:SUB, :], hs[:]
            )

        pending = []

        for m in range(NMACRO):
            # ---- load x^T macro tiles (hi+lo fp16), split across rails ----
            xth = xt_p.tile([128, 32, MACRO], f16, tag="xth")
            for qc in range(4):
                nc.sync.dma_start(
                    xth[:, 8 * qc : 8 * (qc + 1), :],
                    xth_d.ap()[m][:, 8 * qc : 8 * (qc + 1), :],
                )
            xtl = xtl_p.tile([128, 32, MACRO], f16, tag="xtl")
            for qc in range(4):
                nc.scalar.dma_start(
                    xtl[:, 8 * qc : 8 * (qc + 1), :],
                    xtl_d.ap()[m][:, 8 * qc : 8 * (qc + 1), :],
                )

            # ---- router: lam^T [12, MACRO] = W @ x^T (3-pass fp16 hi/lo) ----
            lam_ps = ps_lam.tile([DEPTH, MACRO], f32, tag="lam")
            passes = [(wth_sb, xth), (wth_sb, xtl), (wtl_sb, xth)]
            for p, (wsb, xsb) in enumerate(passes):
                for c in range(32):
                    nc.tensor.matmul(
                        lam_ps[:], wsb[:, c, :], xsb[:, c, :],
                        start=(p == 0 and c == 0), stop=(p == 2 and c == 31),
                    )

            if m == 0:
                # remaining constants, deferred so the first x tile and the
                # router aren't stuck behind the const DMA traffic
                powt_sb = consts.tile([DEPTH, DEPTH], tdt)
                nc.sync.dma_start(powt_sb[:], powt_d.ap())
                offsw_sb = consts.tile([16, DEPTH * (SUB // 16)], f32)
                nc.sync.dma_start(offsw_sb[:], offsw_d.ap())
                bselt_sb = consts.tile([DEPTH, NCHUNK * 128], tdt)
                nc.sync.dma_start(bselt_sb[:], bselt_d.ap())
                nrel_sb = consts.tile([128, NCHUNK], f32)
                nc.sync.dma_start(nrel_sb[:], nrel_d.ap())
                ident_sb = consts.tile([2 * DEPTH, 2 * DEPTH], tdt)
                nc.sync.dma_start(ident_sb[:], ident_d.ap())
                ysh_sb = consts.tile([128, NCHUNK * D], ydt)
                state["ysh"] = ysh_sb
                for c in range(NCHUNK):
                    # full 128 rows: pad slots (>= N_SH) hold valid-but-
                    # unselected Y rows, never NaN-able garbage SBUF
                    lo = c * 128
                    hi = min(lo + 128, N_NODES)
                    nc.scalar.dma_start(
                        ysh_sb[: hi - lo, c * D : (c + 1) * D], y_d.ap()[lo:hi, :]
                    )

            # branch bits, lam^T and prefix^T in SBUF (partition 0 based)
            branch = small.tile([DEPTH, MACRO], tdt, tag="branch")
            nc.vector.tensor_scalar(branch[:], lam_ps[:], 0.0, None, Alu.is_gt)
            lamT = small.tile([DEPTH, MACRO], tdt, tag="lamT")
            nc.scalar.copy(lamT[:], lam_ps[:])

            # prefix^T [12, MACRO] = powmat @ branch  (exact in fp16)
            pb_ps = ps_pb.tile([DEPTH, MACRO], f32, tag="lam")
            nc.tensor.matmul(pb_ps[:], powt_sb[:], branch[:], start=True, stop=True)
            pfxT = small.tile([DEPTH, MACRO], tdt, tag="pfxT")
            nc.scalar.copy(pfxT[:], pb_ps[:])

            # ---- S^T build: one chunk of 128 shallow nodes at a time ----
            st_list = []
            for c in range(NCHUNK):
                bc_ps = ps_bc.tile([128, 2 * MACRO], f32, tag="bc")
                nc.tensor.matmul(
                    bc_ps[:, :MACRO], bselt_sb[:, c * 128 : (c + 1) * 128],
                    pfxT[:], start=True, stop=True,
                )
                nc.tensor.matmul(
                    bc_ps[:, MACRO:], bselt_sb[:, c * 128 : (c + 1) * 128],
                    lamT[:], start=True, stop=True,
                )
                lbc = small.tile([128, MACRO], ydt, tag="lbc")
                nc.scalar.copy(lbc[:], bc_ps[:, MACRO:])
                stc = st_p.tile([128, MACRO], ydt, tag="st")
                nc.vector.scalar_tensor_tensor(
                    stc[:], bc_ps[:, :MACRO], nrel_sb[:, c : c + 1], lbc[:],
                    Alu.is_equal, Alu.mult,
                )
                st_list.append(stc)

            for s in range(NSUB):
                bsl = slice(s * SUB, (s + 1) * SUB)
                # ---- lam to batch-partition layout ----
                # (plain identity matmul: out = in.T @ I)
                tpw = ps_tp.tile([SUB, 128], f32, tag="tpw")
                tp_ps = tpw[:, :DEPTH]
                nc.tensor.matmul(
                    tp_ps, lamT[:, bsl], ident_sb[:DEPTH, :DEPTH],
                    start=True, stop=True,
                )
                lamb = small4.tile([SUB, DEPTH], f32, tag="lamb")
                nc.vector.tensor_copy(lamb[:], tp_ps)

                # ---- node ids in the 16-partition-wrapped (level, slot)
                # layout dma_gather wants, via per-16-column PE transposes ----
                w_ps = tpw[:16, 16 : 16 + (SUB // 16) * DEPTH].rearrange(
                    "p (f l) -> p f l", f=SUB // 16
                )
                for f in range(SUB // 16):
                    nc.tensor.matmul(
                        w_ps[:, f, :],
                        pfxT[:, s * SUB + f * 16 : s * SUB + (f + 1) * 16],
                        ident_sb[:DEPTH, :DEPTH],
                        start=True, stop=True,
                    )
                idx16 = small4.tile([16, DEPTH, SUB // 16], i16, tag="idx16")
                nc.vector.tensor_tensor(
                    idx16[:], w_ps[:].rearrange("p f l -> p l f"), offsw_sb[:],
                    Alu.add,
                )
                # replicate to all 8 Q7 descriptor-gen cores (SBUF -> SBUF)
                idxr = small4.tile([128, N_GL, SUB // 16], i16, tag="idxr")
                for gq in range(8):
                    nc.sync.dma_start(
                        idxr[16 * gq : 16 * (gq + 1), :, :].rearrange(
                            "p l f -> p (l f)"
                        ),
                        idx16[:, K_MM:, :].rearrange("p l f -> p (l f)"),
                    )

                # ---- gather deep levels from HBM ----
                gt = []
                for li in range(N_GL):
                    g = g_p.tile([128, 1, D], ydt, tag="g")
                    nc.gpsimd.dma_gather(
                        g[:], y_d.ap(), idxr[:, li, :], SUB, SUB, D,
                        queue_num=li % 4,
                    )
                    gt.append(g)

                pending.append((m, s, gt, lamb, st_list))
                while len(pending) > PIPE:
                    back_phase(*pending.pop(0))

        while pending:
            back_phase(*pending.pop(0))

    nc.compile()
    return nc


def _patch_walrus_passes():
    # The default walrus pass list in this environment omits
    # lower_custom_kernel, which the Pool custom instructions (dma_gather)
    # need. Inject it in front of codegen.
    import concourse.bass_utils as bu

    if getattr(bu, "_ant_lck_patched", False):
        return
    bu._ant_lck_patched = True
    orig = bu.run_command

    def run_command(argv, **kw):
        if argv and "walrus_driver" in str(argv[0]):
            argv = list(argv)
            for i, a in enumerate(argv):
                if a == "--pass" and "lower_custom_kernel" not in argv[i + 1]:
                    argv[i + 1] = argv[i + 1].replace(
                        "codegen", "lower_custom_kernel,codegen"
                    )
                    break
        return orig(argv, **kw)

    bu.run_command = run_command


def _get_program():
    if "nc" not in _CACHE:
        _CACHE["nc"] = _build_program()
    return _CACHE["nc"]


def _prep_in_maps(x, W, Y):
    powT, offs_w, bselT, nrel, ident = _host_consts()
    Y16 = np.ascontiguousarray(Y.astype(np.float16))
    wt = W.T.reshape(32, 128, DEPTH).transpose(1, 0, 2)  # [128, 32, 12] f32
    wth = wt.astype(np.float16)
    wtl = (wt - wth.astype(np.float32)).astype(np.float16)
    wth = np.ascontiguousarray(wth)
    wtl = np.ascontiguousarray(wtl)
    in_maps = []
    xr = x.reshape(NCORES, B_LOC, D)
    for c in range(NCORES):
        xt = xr[c].T  # [D, B_LOC]
        xtm = xt.reshape(32, 128, NMACRO, MACRO).transpose(2, 1, 0, 3)
        xth = xtm.astype(np.float16)
        xtl = (xtm - xth.astype(np.float32)).astype(np.float16)
        in_maps.append(
            {
                "xth": np.ascontiguousarray(xth),
                "xtl": np.ascontiguousarray(xtl),
                "y": Y16, "wth": wth, "wtl": wtl, "powt": powT,
                "offsw": offs_w, "bselt": bselT, "nrel": nrel, "ident": ident,
            }
        )
    return in_maps


def kernel(x, W, Y, _trace=False):
    from concourse.bass_utils import run_bass_kernel_spmd

    _patch_walrus_passes()

    nc = _get_program()
    in_maps = _prep_in_maps(np.asarray(x), np.asarray(W), np.asarray(Y))
    res = run_bass_kernel_spmd(nc, in_maps, list(range(NCORES)), trace=_trace)
    out = np.concatenate(
        [res.results[c]["out"].astype(np.float32) for c in range(NCORES)], axis=0
    )
    if _trace:
        _CACHE["last_result"] = res
    return out
